# revision 56
# baseline (speedup 1.0000x reference)
"""Trainium2 Bass kernel for LongRangeTCN (4-layer dilated causal conv + BN + LIF + residual).

Sharding: data-parallel over batch B=32 -> 4 per core across 8 NeuronCores.
Per core layout (SBUF, fp32):
  XS  2 x [128, 2, 4112] per-stream residual/input; cols [0,16) zero pad
                      (conv halo), col 16+t = x_t
  XH  [128, 4, 4160]  scan input/trajectory; cols [0,64) zero (warmup), col 64+t holds
                      xh_t = 0.5*BN(conv(x))_t, overwritten in-place by A_t during the scan
  WT/WL [128, 4, 3, 128] folded conv weights split as fp32r-exact hi + remainder
  BIAS [128, 4]        folded BN bias (per-channel) * 0.5

Conv: fp32r matmuls (4x PE throughput vs fp32; fp32r rounds inputs to 11
mantissa bits, so W is split host-side into Wh + Wl and both accumulated,
leaving only activation rounding; the pipeline-gating layer0/stream0 conv
skips the Wl term). PE is pre-warmed with dummy matmuls so convs don't run
at the cold p-state. Matmuls accumulate in PSUM; ACT evacuates adding the
BN bias. Input/output DMAs are quartered so compute streams behind them.

LIF scan: v' = (A<1)*A with A = 0.5*v + xh_t, as 128 parallel chunks of 32
steps per batch with an 11-step warmup (conv fp32r noise ~1e-4 dominates the
0.5^11 chunk-carry error). Chunks are split into two interleaved groups so
consecutive DVE ops are independent: the engine's ~95ns write-to-read drain
(which hardware requires a semaphore wait for — unsynced dependent DVE ops
return garbage) lands while the other group's op executes. The scan state
lives in raw SBUF outside tile pools; the one cross-engine edge into it
(ACT evict -> DVE scan) is a manual semaphore. Spikes s=(A>=1) + residual
are fused ops; at the last layer stream 0's spike moves to the host (its
A trajectory and pre-spike residual stream out under cover of stream 1's
scan) and stream 1's is split so output DMA overlaps. Layer 3 skips the
Wl term (its flips don't cascade).
"""

import numpy as np

TAU, VTH, EPS, K = 2.0, 1.0, 1e-5, 3
DILATIONS = (1, 2, 4, 8)
B, C, T = 32, 128, 4096
NCORES = 8
BL = B // NCORES          # 4 batches per core
H = 11                    # scan warmup steps
LSC = 32                  # scan chunk length
NC2 = T // LSC            # 128 chunks per batch
NG = NC2 // 2             # chunks per interleave group
PAD0 = 64                 # zero-pad columns at the head of each batch row in XH
OFF = PAD0 - H            # 52: step j of chunk c touches col c*LSC + OFF + j
PADX = 16                 # conv left halo (max (K-1)*d = 16)
SX = PADX + T             # 4112
SXH = PAD0 + T            # 4160 = 130*32
NS = 2                    # batch streams per core (pipeline conv under scan)
BS = BL // NS             # 2 batches per stream

_cache = {}


def _build():
    import concourse.bass as bass
    import concourse.bacc as bacc
    import concourse.tile as tile
    import concourse.mybir as mybir

    dt = mybir.dt.float32
    f32r = mybir.dt.float32r
    Alu = mybir.AluOpType
    Act = mybir.ActivationFunctionType

    nc = bacc.Bacc("TRN2", target_bir_lowering=False, debug=False)
    x_d = nc.dram_tensor("x", [BL, C, T], dt, kind="ExternalInput")
    wt_d = nc.dram_tensor("wt", [C, 4, K, C], dt, kind="ExternalInput")
    wl_d = nc.dram_tensor("wl", [C, 4, K, C], dt, kind="ExternalInput")
    b_d = nc.dram_tensor("bias", [C, 4], dt, kind="ExternalInput")
    o_d = nc.dram_tensor("out", [BL, C, T], dt, kind="ExternalOutput")
    # stream 0's layer-3 A trajectory; its final spike+residual runs on the
    # host (out[b] = (A>=1) + X3), freeing DVE time with no store-tail cost
    ao_d = nc.dram_tensor("outa", [BS, C, T], dt, kind="ExternalOutput")

    # Scan state lives outside tile pools; the only cross-engine edge into it
    # (ACT evict -> DVE scan) gets one manual semaphore per stream/layer.
    XH = [nc.alloc_sbuf_tensor(f"XHraw{s}", [C, BS, SXH], dt).ap() for s in range(NS)]
    V = [nc.alloc_sbuf_tensor(f"Vraw{s}", [C, BS, NC2], dt).ap() for s in range(NS)]
    SCR = [nc.alloc_sbuf_tensor(f"SCRraw{s}", [C, BS, NC2], dt).ap() for s in range(NS)]
    WU = nc.alloc_sbuf_tensor("WUraw", [C, 640], dt).ap()  # PE warmup garbage
    ev_sem = nc.alloc_semaphore("evict_done")
    sc_sem = nc.alloc_semaphore("scan30_done")

    with tile.TileContext(nc) as tc:
        with (
            tc.tile_pool(name="big", bufs=1) as big,
            tc.tile_pool(name="small", bufs=1) as small,
            tc.tile_pool(name="psum", bufs=4, space="PSUM") as pp,
        ):
            # one X tile per stream: keeps spike writes of one stream from
            # false-serializing against conv reads of the other
            XS = [big.tile([C, BS, SX], dt, name=f"X{s}", tag=f"X{s}") for s in range(NS)]
            WT = small.tile([C, 4, K, C], dt, tag="WT")
            WL = small.tile([C, 4, K, C], dt, name="WL", tag="WL")
            BIAS = small.tile([C, 4], dt, tag="BIAS")

            # p-state warmup: keep PE continuously busy through the input DMA
            # window so real convs start at full clock.
            wups = pp.tile([C, 512], dt, tag="wup")
            for _ in range(20):
                nc.tensor.matmul(wups[:], WU[:, 0:128].bitcast(f32r),
                                 WU[:, 128:640].bitcast(f32r), start=True, stop=True)
            # trigger the activation-table load (~1.3us) during the DMA window
            # so the first real evict doesn't pay it
            nc.scalar.activation(SCR[0][:, 0:1, 0:1], WU[:, 1:2], Act.Identity,
                                 bias=0.0, scale=1.0)

            # layer-0 Wh first so only it gates the first conv (the head conv
            # skips the Wl term entirely)
            nc.sync.dma_start(WT[:, 0].bitcast(f32r), wt_d[:, 0].bitcast(f32r))
            nc.sync.dma_start(BIAS[:], b_d[:])
            for st in range(NS):
                nc.vector.memset(XS[st][:, :, 0:PADX], 0.0)
            for st in range(NS):
                nc.vector.memset(XH[st][:, :, 0:PAD0], 0.0)
            for s in range(NS):
                nc.vector.memset(V[s], 0.0)
            TQ = T // 4
            for b in range(BL):
                for hh in range(4):
                    nc.sync.dma_start(
                        XS[b // BS][:, b % BS, PADX + hh * TQ : PADX + (hh + 1) * TQ].bitcast(f32r),
                        x_d[b][:, hh * TQ : (hh + 1) * TQ].bitcast(f32r))
                if b == 1:
                    # remaining weights after stream 0's activations
                    nc.sync.dma_start(WT[:, 1:4].bitcast(f32r), wt_d[:, 1:4].bitcast(f32r))
                    nc.sync.dma_start(WL[:].bitcast(f32r), wl_d[:].bitcast(f32r))

            XH4 = [XH[st].rearrange("p a (c l) -> p a c l", l=LSC) for st in range(NS)]
            ev_cnt = [0]

            def conv(li, d, s):
                # layer0/stream0 gates the whole pipeline: skip the Wl
                # compensation there (halves the critical-path PE time for a
                # ~5% bump in flip count on that one stream-layer)
                comp = not (li == 0 and s == 0) and li != 3
                for b in range(s * BS, (s + 1) * BS):
                    for t0 in range(0, T, 512):
                        ps = pp.tile([C, 512], dt, tag="ps")
                        for k in range(K):
                            sh = (K - 1 - k) * d
                            rhs = XS[s][:, b - s * BS, PADX + t0 - sh : PADX + t0 - sh + 512].bitcast(f32r)
                            nc.tensor.matmul(ps[:], WT[:, li, k, :].bitcast(f32r), rhs,
                                             start=(k == 0), stop=(k == K - 1) and not comp)
                            if comp:
                                nc.tensor.matmul(ps[:], WL[:, li, k, :].bitcast(f32r), rhs,
                                                 start=False, stop=(k == K - 1))
                        nc.scalar.activation(
                            XH[s][:, b - s * BS, PAD0 + t0 : PAD0 + t0 + 512], ps[:],
                            Act.Identity, bias=BIAS[:, li : li + 1], scale=1.0,
                        )
                nc.scalar.sem_inc(ev_sem)
                ev_cnt[0] += 1

            def scan(s):
                b0 = s * BS
                last = H + LSC - 1
                for j in range(H + LSC):
                    # absolute col of step j in chunk c is c*LSC + OFF + j;
                    # OFF+j spans [52, 96) so the chunk-view offset is 1 or 2
                    qo, r = divmod(OFF + j, LSC)
                    cols = [XH4[s][:, :, g * NG + qo : (g + 1) * NG + qo, r]
                            for g in range(2)]
                    ads = [SCR[s][:, :, g * NG : (g + 1) * NG] if j < H else cols[g]
                           for g in range(2)]
                    vs = [V[s][:, :, g * NG : (g + 1) * NG] for g in range(2)]
                    for g in range(2):
                        # A = 0.5*v + xh_t (overwrites xh col in place when j>=H)
                        op = nc.vector.scalar_tensor_tensor(
                            ads[g], vs[g], 0.5, cols[g], op0=Alu.mult, op1=Alu.add)
                        if j == 0 and g == 0:
                            op.wait_op(ev_sem, ev_cnt[0], "sem-ge")
                    # v' = (A < 1) * A; the final step's state is never used
                    if j != last:
                        for g in range(2):
                            nc.vector.scalar_tensor_tensor(
                                vs[g], ads[g], float(VTH), ads[g],
                                op0=Alu.is_lt, op1=Alu.mult)

            def spike_res(s, t0, t1):
                b0 = s * BS
                nc.vector.scalar_tensor_tensor(
                    XS[s][:, :, PADX + t0 : PADX + t1].bitcast(f32r),
                    XH[s][:, :, PAD0 + t0 : PAD0 + t1], float(VTH),
                    XS[s][:, :, PADX + t0 : PADX + t1],
                    op0=Alu.is_ge, op1=Alu.add)

            # software pipeline: stream s+1's conv (PE) overlaps stream s's
            # scan (DVE); across layers likewise — Tile schedules by deps.
            for li, d in enumerate(DILATIONS):
                for s in range(NS):
                    conv(li, d, s)
                    if li == len(DILATIONS) - 1 and s == 0:
                        # stream 0's final spike happens on the host: store the
                        # pre-spike residual now (hidden under its own scan)...
                        for hh in range(4):
                            q0, q1 = hh * (T // 4), (hh + 1) * (T // 4)
                            for b in range(BS):
                                nc.sync.dma_start(
                                    o_d[b][:, q0:q1],
                                    XS[0][:, b, PADX + q0 : PADX + q1])
                    scan(s)
                    if li < len(DILATIONS) - 1:
                        # bias the scheduler to slot the spike right after its
                        # scan, ahead of the next layer's evict-blocked scan
                        with tc.high_priority(offset=400):
                            spike_res(s, 0, T)
                    elif s == 0:
                        # ...and the A trajectory after the scan (hidden under
                        # stream 1's scan). Raw XH is untracked: manual sem.
                        nc.vector.sem_inc(sc_sem)
                        first = True
                        for hh in range(4):
                            q0, q1 = hh * (T // 4), (hh + 1) * (T // 4)
                            for b in range(BS):
                                dma = nc.sync.dma_start(
                                    ao_d[b][:, q0:q1],
                                    XH[0][:, b, PAD0 + q0 : PAD0 + q1])
                                if first:
                                    dma.wait_op(sc_sem, 1, "sem-ge")
                                    first = False
                    else:
                        # stream 1 keeps the device spike: its A-store tail
                        # would cost more than the spike saves
                        for hh in range(8):
                            q0, q1 = hh * (T // 8), (hh + 1) * (T // 8)
                            spike_res(s, q0, q1)
                            for b in range(s * BS, (s + 1) * BS):
                                nc.sync.dma_start(
                                    o_d[b][:, q0:q1],
                                    XS[s][:, b - s * BS, PADX + q0 : PADX + q1])

    nc.clear_and_free_semaphores([ev_sem, sc_sem])
    nc.all_engine_barrier()
    nc.compile()
    return nc


def _round11(a):
    """Round fp32 to 11 explicit mantissa bits (fp32r's internal rounding)."""
    u = np.asarray(a, np.float32).view(np.uint32)
    return ((u + np.uint32(1 << 11)) & np.uint32(0xFFFFF000)).view(np.float32)


def kernel(x, w, gamma, beta, mean, var, **_):
    from concourse.bass_utils import run_bass_kernel_spmd

    x = np.ascontiguousarray(x, np.float32)
    inv = (gamma / np.sqrt(var + EPS)).astype(np.float32)          # [4, C]
    # wt[ci, l, k, co] = 0.5 * w[l, co, ci, k] * inv[l, co]
    wt = (0.5 * w * inv[:, :, None, None]).astype(np.float32)      # [4, Co, Ci, K]
    wt = np.ascontiguousarray(wt.transpose(2, 0, 3, 1))            # [Ci, 4, K, Co]
    wh = _round11(wt)                                              # exact under fp32r
    wl = np.ascontiguousarray(wt - wh)                             # remainder term
    bias = (0.5 * (beta - mean * inv)).astype(np.float32).T        # [C, 4]
    bias = np.ascontiguousarray(bias)

    if "nc" not in _cache:
        _cache["nc"] = _build()
    nc = _cache["nc"]

    in_maps = [
        {"x": np.ascontiguousarray(x[i * BL : (i + 1) * BL]), "wt": wh, "wl": wl,
         "bias": bias}
        for i in range(NCORES)
    ]
    res = run_bass_kernel_spmd(nc, in_maps, list(range(NCORES)))
    outs = []
    for i in range(NCORES):
        o = np.array(res.results[i]["out"])
        a = res.results[i]["outa"]
        # stream 0's final spike+residual, applied host-side
        o[0:BS] += (a >= VTH).astype(np.float32)
        outs.append(o)
    return np.concatenate(outs, axis=0)
